# revision 3
# baseline (speedup 1.0000x reference)
"""Multi-head attention (B=4, T=2048, D=1024, H=16) on 8 TRN2 NeuronCores.

Sharding: core c -> (batch b = c//2, head-group g = c%2 of 8 heads).
Each core computes the qkv projection for its batch restricted to its 8
heads, full attention for those heads, and a partial output projection
(ctx_local @ Wout[rows of its heads]).  Host sums the two partials per batch.

v2: flat software pipeline.  x is resident in SBUF (loaded once).  The
attention runs as one uniform 256-step stream over (pair, quarter, kchunk):
per step  [S-pair(i) | exp(i-1) on ACT | extra proj work | AV-pair(i-2)],
with all projection matmuls (q/k/v/out) spread into the per-step "extra"
slots so ACT (the softmax-exp engine, the throughput floor at ~1.1us per
[128,1024] exp) stays saturated from ~10us onward.  Startup DMAs are
chunked (x span 0 + pair-0 weights first, on two HWDGE queues) so the
first S matmul issues at ~9us.  Quarter-end ctx PSUM banks are freed by a
fast DVE copy to SBUF scratch before the (slow) reciprocal+broadcast
normalization, so the next quarter's AV never stalls on the norm chain.
"""

import numpy as np
import ml_dtypes
from contextlib import ExitStack

import concourse.bass as bass
import concourse.bacc as bacc
import concourse.tile as tile
from concourse import mybir
from concourse.bass_utils import run_bass_kernel_spmd

FP32 = mybir.dt.float32
BF16 = mybir.dt.bfloat16
EXP = mybir.ActivationFunctionType.Exp

D = 1024
T = 2048
FC = 8      # feature chunks of 128 (projection contraction)
KC = 16     # k chunks of 128
QQ = 4      # query quarters of 512
NPAIR = 4   # head pairs per core
RING = 8    # P2 ring depth (k-chunk slots)
NSTEP = NPAIR * QQ * KC  # 256


def _body(ctx, nc, tc, xt_d, wq_d, wk_d, wv_d, wo_d, out_d):
    per = ctx.enter_context(tc.tile_pool(name="persist", bufs=1))
    xt_sb = per.tile([128, FC, T], BF16, tag="xt")
    qT = per.tile([128, NPAIR, T], BF16, tag="qT")
    kT = per.tile([128, NPAIR, T], BF16, tag="kT")
    ctx_sb = per.tile([128, NPAIR, T], BF16, tag="ctx")
    v_sb = per.tile([128, KC, 8, 65], BF16, tag="v")
    wq_sb = per.tile([128, FC, 512], BF16, tag="wq")
    wk_sb = per.tile([128, FC, 512], BF16, tag="wk")
    wv_sb = per.tile([128, FC, 512], BF16, tag="wv")
    wo_sb = per.tile([128, 4, D], BF16, tag="wo")
    P2 = per.tile([128, RING, 2, 512], BF16, tag="P2")
    warm_in = per.tile([1, 16], FP32, tag="warm_in")
    warm_out = per.tile([1, 16], FP32, tag="warm_out")

    xt_r = xt_d.rearrange("(f p) t -> p f t", p=128)
    wq_r = wq_d.rearrange("(f p) c -> p f c", p=128)
    wk_r = wk_d.rearrange("(f p) c -> p f c", p=128)
    wv_r = wv_d.rearrange("(f p) c -> p f c", p=128)
    wo_r = wo_d.rearrange("(c p) d -> p c d", p=128)

    # --- prologue DMAs.  sync queue: x spans + background weights;
    # scalar queue (ACT is idle until the first exp): pair-0 q/k weights.
    nc.sync.dma_start(out=xt_sb[:, :, 0:512], in_=xt_r[:, :, 0:512])
    nc.scalar.dma_start(out=wk_sb[:, :, 0:128], in_=wk_r[:, :, 0:128])
    nc.scalar.dma_start(out=wq_sb[:, :, 0:128], in_=wq_r[:, :, 0:128])
    nc.sync.dma_start(out=wv_sb[:, :, 0:128], in_=wv_r[:, :, 0:128])
    nc.scalar.dma_start(out=xt_sb[:, :, 1536:2048], in_=xt_r[:, :, 1536:2048])
    for s in range(1, 3):
        nc.sync.dma_start(
            out=xt_sb[:, :, s * 512:(s + 1) * 512],
            in_=xt_r[:, :, s * 512:(s + 1) * 512])
    nc.sync.dma_start(out=wk_sb[:, :, 128:512], in_=wk_r[:, :, 128:512])
    nc.sync.dma_start(out=wq_sb[:, :, 128:512], in_=wq_r[:, :, 128:512])
    nc.sync.dma_start(out=wv_sb[:, :, 128:512], in_=wv_r[:, :, 128:512])
    nc.sync.dma_start(out=wo_sb[:], in_=wo_r)

    # ones row for the fused sumexp; warm the ACT exp table during the DMAs
    nc.vector.memset(v_sb[:, :, :, 64:65], 1.0)
    nc.vector.memset(warm_in[:], 0.0)
    nc.scalar.activation(out=warm_out[:], in_=warm_in[:], func=EXP, scale=1.0)

    ps = ctx.enter_context(tc.tile_pool(name="ps", bufs=1, space="PSUM"))
    psv = ctx.enter_context(tc.tile_pool(name="psv", bufs=1, space="PSUM"))
    spsum = ctx.enter_context(tc.tile_pool(name="sps", bufs=2, space="PSUM"))
    cpsum = ctx.enter_context(tc.tile_pool(name="cps", bufs=2, space="PSUM"))
    scr = ctx.enter_context(tc.tile_pool(name="scr", bufs=2))
    rpool = ctx.enter_context(tc.tile_pool(name="rp", bufs=2))
    osb = ctx.enter_context(tc.tile_pool(name="osb", bufs=2))

    # ---------------- extra-work generators (thunks emit on call) ----------
    # Chains that share a PSUM pool must be strictly sequential (pool buffers
    # cycle per tile() call; interleaved live chains collide).  ps and psv
    # are two single-bank pools used as a manual double buffer: concurrent
    # chain families get one each, and op units alternate halves across both.
    def g_proj(which, pair, ts, pool):
        """q or k projection of one 512-token span for one head pair:
        8 accumulating matmuls + 1 copy-out, yielded as 9 thunks."""
        w_sb, dst = (wq_sb, qT) if which == "q" else (wk_sb, kT)
        tsl = slice(ts * 512, (ts + 1) * 512)
        st = {}

        def mk(fc):
            def th():
                if fc == 0:
                    st["p"] = pool.tile([128, 512], FP32, tag="proj", name="qk")
                nc.tensor.matmul(
                    st["p"][:],
                    lhsT=w_sb[:, fc, pair * 128:(pair + 1) * 128],
                    rhs=xt_sb[:, fc, tsl],
                    start=(fc == 0), stop=(fc == FC - 1))
            return th

        for fc in range(FC):
            yield mk(fc)

        def cp():
            nc.vector.tensor_copy(out=dst[:, pair, tsl], in_=st["p"][:])
        yield cp

    def g_vp2(pairlo, kc, pool):
        """v-projection of two adjacent head pairs (256 v-dims) for one
        128-token chunk: 8 matmuls (256-col streams) + 1 copy."""
        st = {}

        def mk(fc):
            def th():
                if fc == 0:
                    st["p"] = pool.tile([128, 512], FP32, tag="proj", name="vp")
                nc.tensor.matmul(
                    st["p"][:, 0:256],
                    lhsT=xt_sb[:, fc, kc * 128:(kc + 1) * 128],
                    rhs=wv_sb[:, fc, pairlo * 128:(pairlo + 2) * 128],
                    start=(fc == 0), stop=(fc == FC - 1))
            return th

        for fc in range(FC):
            yield mk(fc)

        def cp():
            nc.vector.tensor_copy(
                out=v_sb[:, kc, 2 * pairlo:2 * pairlo + 4, 0:64],
                in_=st["p"][:, 0:256].rearrange("p (h d) -> p h d", h=4))
        yield cp

    op_half = [0]

    def g_op(q):
        """output projection of quarter q's tokens: 4 token chunks; per
        chunk the two 512-column halves accumulate in alternating ps/psv
        banks (ping-pong across the whole op stream) with their matmuls
        interleaved cc-major, so the cc<3 accumulation runs before the
        final quarter's norm lands and same-bank stalls are avoided."""
        st = {}

        def mk(u, j2, cc):
            def th():
                tcg = q * 4 + u
                if cc == 0:
                    if j2 == 0:
                        st["ot"] = osb.tile([128, D], BF16, tag="ot", name="ot")
                    op_half[0] += 1
                    st[j2] = (ps if op_half[0] % 2 == 0 else psv).tile(
                        [128, 512], FP32, tag="proj", name="po")
                nc.tensor.matmul(
                    st[j2][:],
                    lhsT=ctx_sb[:, cc, tcg * 128:(tcg + 1) * 128],
                    rhs=wo_sb[:, cc, j2 * 512:(j2 + 1) * 512],
                    start=(cc == 0), stop=(cc == 3))
            return th

        def mkcp(u, j2):
            def th():
                tcg = q * 4 + u
                nc.vector.tensor_copy(
                    out=st["ot"][:, j2 * 512:(j2 + 1) * 512], in_=st[j2][:])
                if j2 == 1:
                    eng = nc.scalar if (q == 3 and u % 2 == 1) else nc.sync
                    eng.dma_start(
                        out=out_d[tcg * 128:(tcg + 1) * 128, :],
                        in_=st["ot"][:])
            return th

        for u in range(4):
            for cc in range(3):
                for j2 in range(2):
                    yield mk(u, j2, cc)
            yield mk(u, 0, 3)
            yield mkcp(u, 0)
            yield mk(u, 1, 3)
            yield mkcp(u, 1)

    # ---------------- schedule: spread thunks over step ranges -------------
    sched = [[] for _ in range(NSTEP + 2)]

    def spread(thunks, s0, s1):
        thunks = list(thunks)
        n, span = len(thunks), s1 - s0
        for idx, th in enumerate(thunks):
            sched[s0 + (idx * span) // n].append(th)

    def chain(*gens):
        for g in gens:
            yield from g

    # ps chain, steps 0-16: k spans 1-3 of pair 0 (span s needed by step
    # 4s) then q span 1 (needed by step 16) — one sequential chain.
    spread(chain(g_proj("k", 0, 1, ps), g_proj("k", 0, 2, ps),
                 g_proj("k", 0, 3, ps), g_proj("q", 0, 1, ps)), 0, 16)
    # psv chain, steps 0-15: v pairs 0+1, chunk kc at step kc (consumed at
    # step kc+2 / from step 66 for pair 1)
    for kc in range(KC):
        spread(g_vp2(0, kc, psv), kc, kc + 1)
    # ps chain, steps 16-64: pair-0 q spans 2,3 then all of pair 1's q/k
    spread(chain(g_proj("q", 0, 2, ps), g_proj("q", 0, 3, ps),
                 *[g_proj(w, 1, ts, ps) for w in ("k", "q") for ts in range(4)]),
           16, 64)
    # psv chain, steps 16-64: v pairs 2+3 (needed from step 128)
    spread(chain(*[g_vp2(2, kc, psv) for kc in range(KC)]), 16, 64)
    # pair 2 / pair 3 q/k projections: units alternate ps/psv banks
    for base, pair in ((64, 2), (128, 3)):
        units = [g_proj(w, pair, ts, (ps if i % 2 == 0 else psv))
                 for i, (w, ts) in enumerate(
                     (w, ts) for w in ("k", "q") for ts in range(4))]
        spread(chain(*units), base, base + 64)
    # output projection of hc3's quarters 0-2 during quarters 1-3.
    # norm(3,q) is emitted at step 209+16q (with the AV of step a=207+16q),
    # so op(q) must not enter the PE stream before step 210+16q or it
    # head-of-line-blocks the PE behind a dependency on a later AV.
    for q in range(3):
        spread(g_op(q), 210 + 16 * q, min(226 + 16 * q, NSTEP))

    # ---------------- prologue projections (k span 0, q span 0, pair 0) ----
    # interleave the two chains (ps/psv banks) to avoid same-bank
    # accumulation stalls in the serial prologue
    for thk, thq in zip(g_proj("k", 0, 0, ps), g_proj("q", 0, 0, psv)):
        thk()
        thq()

    # ---------------- normalization at quarter end -------------------------
    def norm_q(pc, qq, ctxA, ctxB):
        qsl = slice(qq * 512, (qq + 1) * 512)
        # fast copies free the ctx PSUM banks for the next quarter's AV; the
        # Z rows must land at partition 0 (custom-DVE reciprocal reads the
        # input AP without its partition offset).
        sc = scr.tile([64, 2, 512], FP32, tag="scr", name="scr")
        zc = rpool.tile([1, 2, 512], FP32, tag="zc", name="zc")
        nc.vector.tensor_copy(out=sc[:, 0, :], in_=ctxA[0:64, :])
        nc.vector.tensor_copy(out=zc[:, 0, :], in_=ctxA[64:65, :])
        nc.vector.tensor_copy(out=sc[:, 1, :], in_=ctxB[0:64, :])
        nc.vector.tensor_copy(out=zc[:, 1, :], in_=ctxB[64:65, :])
        rt = rpool.tile([1, 2, 512], FP32, tag="rt", name="rt")
        nc.vector.reciprocal_approx_fast(out=rt[:], in_=zc[:])
        for h in (0, 1):
            rb = rpool.tile([64, 512], FP32, tag="rb")
            nc.gpsimd.partition_broadcast(rb[:], rt[0:1, h, :], channels=64)
            nc.vector.tensor_mul(
                ctx_sb[h * 64:h * 64 + 64, pc, qsl], sc[:, h, :], rb[:])

    # ---------------- main pipeline ----------------------------------------
    sps_hist = {}
    ctxA = ctxB = None
    for i in range(NSTEP + 2):
        if i < NSTEP:
            pc, r = divmod(i, QQ * KC)
            qq, kc = divmod(r, KC)
            qsl = slice(qq * 512, (qq + 1) * 512)
            sps = spsum.tile([128, 2, 512], FP32, tag="S", name="S")
            for h in (0, 1):
                b0 = h * 64
                nc.tensor.matmul(
                    sps[:, h, :],
                    lhsT=kT[b0:b0 + 64, pc, kc * 128:(kc + 1) * 128],
                    rhs=qT[b0:b0 + 64, pc, qsl],
                    start=True, stop=True)
            sps_hist[i] = sps
        j = i - 1
        if 0 <= j < NSTEP:
            nc.scalar.activation(
                out=P2[:, j % RING, :, :], in_=sps_hist.pop(j)[:, :, :],
                func=EXP, scale=0.125)
        for th in sched[i]:
            th()
        a = i - 2
        if 0 <= a < NSTEP:
            apc, ar = divmod(a, QQ * KC)
            aqq, akc = divmod(ar, KC)
            if akc == 0:
                ctxA = cpsum.tile([65, 512], FP32, tag="ctx", name="cA")
                ctxB = cpsum.tile([65, 512], FP32, tag="ctx", name="cB")
            for hi, ctp in ((0, ctxA), (1, ctxB)):
                nc.tensor.matmul(
                    ctp[:],
                    lhsT=v_sb[:, akc, 2 * apc + hi, :],
                    rhs=P2[:, a % RING, hi, :],
                    start=(akc == 0), stop=(akc == KC - 1))
            if akc == KC - 1:
                norm_q(apc, aqq, ctxA, ctxB)

    # ---------------- tail: output projection of the last quarter ----------
    for th in g_op(3):
        th()
    return qT, kT, v_sb, ctx_sb


def build():
    nc = bacc.Bacc("TRN2", target_bir_lowering=False, debug=False, num_devices=8)
    xt_d = nc.dram_tensor("xt", [D, T], BF16, kind="ExternalInput").ap()
    wq_d = nc.dram_tensor("wq", [D, 512], BF16, kind="ExternalInput").ap()
    wk_d = nc.dram_tensor("wk", [D, 512], BF16, kind="ExternalInput").ap()
    wv_d = nc.dram_tensor("wv", [D, 512], BF16, kind="ExternalInput").ap()
    wo_d = nc.dram_tensor("wout", [512, D], BF16, kind="ExternalInput").ap()
    out_d = nc.dram_tensor("out", [T, D], BF16, kind="ExternalOutput").ap()
    with tile.TileContext(nc) as tc:
        with ExitStack() as ctx:
            _body(ctx, nc, tc, xt_d, wq_d, wk_d, wv_d, wo_d, out_d)
    nc.compile()
    return nc


_nc = None


def _get_nc():
    global _nc
    if _nc is None:
        _nc = build()
    return _nc


def make_in_maps(x, Wqkv, Wout):
    bf = ml_dtypes.bfloat16
    in_maps = []
    for c in range(8):
        b, g = divmod(c, 2)
        cs = slice(g * 512, (g + 1) * 512)
        in_maps.append({
            "xt": np.ascontiguousarray(x[b].T).astype(bf),
            "wq": np.ascontiguousarray(Wqkv[:, 0 * D:1 * D][:, cs]).astype(bf),
            "wk": np.ascontiguousarray(Wqkv[:, 1 * D:2 * D][:, cs]).astype(bf),
            "wv": np.ascontiguousarray(Wqkv[:, 2 * D:3 * D][:, cs]).astype(bf),
            "wout": np.ascontiguousarray(Wout[cs, :]).astype(bf),
        })
    return in_maps


def kernel(x, Wqkv, Wout, _trace=False):
    nc = _get_nc()
    x = np.asarray(x, dtype=np.float32)
    Wqkv = np.asarray(Wqkv, dtype=np.float32)
    Wout = np.asarray(Wout, dtype=np.float32)
    in_maps = make_in_maps(x, Wqkv, Wout)
    kwargs = {}
    if _trace:
        kwargs["trace"] = True
    res = run_bass_kernel_spmd(nc, in_maps, core_ids=list(range(8)), **kwargs)
    outs = [np.asarray(res.results[c]["out"], dtype=np.float32) for c in range(8)]
    out = np.stack([outs[2 * b] + outs[2 * b + 1] for b in range(4)])
    if _trace:
        kernel.last_result = res
    return out


# revision 4
# speedup vs baseline: 1.0018x; 1.0018x over previous
"""Multi-head attention (B=4, T=2048, D=1024, H=16) on 8 TRN2 NeuronCores.

Sharding: core c -> (batch b = c//2, head-group g = c%2 of 8 heads).
Each core computes the qkv projection for its batch restricted to its 8
heads, full attention for those heads, and a partial output projection
(ctx_local @ Wout[rows of its heads]).  Host sums the two partials per batch.

v2: flat software pipeline.  x is resident in SBUF (loaded once).  The
attention runs as one uniform 256-step stream over (pair, quarter, kchunk):
per step  [S-pair(i) | exp(i-1) on ACT | extra proj work | AV-pair(i-2)],
with all projection matmuls (q/k/v/out) spread into the per-step "extra"
slots so ACT (the softmax-exp engine, the throughput floor at ~1.1us per
[128,1024] exp) stays saturated from ~10us onward.  Startup DMAs are
chunked (x span 0 + pair-0 weights first, on two HWDGE queues) so the
first S matmul issues at ~9us.  Quarter-end ctx PSUM banks are freed by a
fast DVE copy to SBUF scratch before the (slow) reciprocal+broadcast
normalization, so the next quarter's AV never stalls on the norm chain.
"""

import numpy as np
import ml_dtypes
from contextlib import ExitStack

import concourse.bass as bass
import concourse.bacc as bacc
import concourse.tile as tile
from concourse import mybir
from concourse.bass_utils import run_bass_kernel_spmd

FP32 = mybir.dt.float32
BF16 = mybir.dt.bfloat16
EXP = mybir.ActivationFunctionType.Exp

D = 1024
T = 2048
FC = 8      # feature chunks of 128 (projection contraction)
KC = 16     # k chunks of 128
QQ = 4      # query quarters of 512
NPAIR = 4   # head pairs per core
RING = 8    # P2 ring depth (k-chunk slots)
NSTEP = NPAIR * QQ * KC  # 256


def _body(ctx, nc, tc, xt_d, wq_d, wk_d, wv_d, wo_d, out_d):
    per = ctx.enter_context(tc.tile_pool(name="persist", bufs=1))
    xt_sb = per.tile([128, FC, T], BF16, tag="xt")
    qT = per.tile([128, NPAIR, T], BF16, tag="qT")
    kT = per.tile([128, NPAIR, T], BF16, tag="kT")
    ctx_sb = per.tile([128, NPAIR, T], BF16, tag="ctx")
    v_sb = per.tile([128, KC, 8, 65], BF16, tag="v")
    wq_sb = per.tile([128, FC, 512], BF16, tag="wq")
    wk_sb = per.tile([128, FC, 512], BF16, tag="wk")
    wv_sb = per.tile([128, FC, 512], BF16, tag="wv")
    wo_sb = per.tile([128, 4, D], BF16, tag="wo")
    P2 = per.tile([128, RING, 2, 512], BF16, tag="P2")
    warm_in = per.tile([1, 16], FP32, tag="warm_in")
    warm_out = per.tile([1, 16], FP32, tag="warm_out")

    xt_r = xt_d.rearrange("(f p) t -> p f t", p=128)
    wq_r = wq_d.rearrange("(f p) c -> p f c", p=128)
    wk_r = wk_d.rearrange("(f p) c -> p f c", p=128)
    wv_r = wv_d.rearrange("(f p) c -> p f c", p=128)
    wo_r = wo_d.rearrange("(c p) d -> p c d", p=128)

    # --- prologue DMAs.  sync queue: x spans + background weights;
    # scalar queue (ACT is idle until the first exp): pair-0 q/k weights.
    nc.sync.dma_start(out=xt_sb[:, :, 0:512], in_=xt_r[:, :, 0:512])
    nc.scalar.dma_start(out=wk_sb[:, :, 0:128], in_=wk_r[:, :, 0:128])
    nc.scalar.dma_start(out=wq_sb[:, :, 0:128], in_=wq_r[:, :, 0:128])
    nc.sync.dma_start(out=wv_sb[:, :, 0:128], in_=wv_r[:, :, 0:128])
    nc.scalar.dma_start(out=xt_sb[:, :, 1536:2048], in_=xt_r[:, :, 1536:2048])
    for s in range(1, 3):
        nc.sync.dma_start(
            out=xt_sb[:, :, s * 512:(s + 1) * 512],
            in_=xt_r[:, :, s * 512:(s + 1) * 512])
    nc.sync.dma_start(out=wk_sb[:, :, 128:512], in_=wk_r[:, :, 128:512])
    nc.sync.dma_start(out=wq_sb[:, :, 128:512], in_=wq_r[:, :, 128:512])
    nc.sync.dma_start(out=wv_sb[:, :, 128:512], in_=wv_r[:, :, 128:512])
    nc.sync.dma_start(out=wo_sb[:], in_=wo_r)

    # ones row for the fused sumexp; warm the ACT exp table during the DMAs
    nc.vector.memset(v_sb[:, :, :, 64:65], 1.0)
    nc.vector.memset(warm_in[:], 0.0)
    nc.scalar.activation(out=warm_out[:], in_=warm_in[:], func=EXP, scale=1.0)

    ps = ctx.enter_context(tc.tile_pool(name="ps", bufs=1, space="PSUM"))
    psv = ctx.enter_context(tc.tile_pool(name="psv", bufs=1, space="PSUM"))
    spsum = ctx.enter_context(tc.tile_pool(name="sps", bufs=2, space="PSUM"))
    cpsum = ctx.enter_context(tc.tile_pool(name="cps", bufs=2, space="PSUM"))
    scr = ctx.enter_context(tc.tile_pool(name="scr", bufs=2))
    rpool = ctx.enter_context(tc.tile_pool(name="rp", bufs=2))
    osb = ctx.enter_context(tc.tile_pool(name="osb", bufs=2))

    # ---------------- extra-work generators (thunks emit on call) ----------
    # Chains that share a PSUM pool must be strictly sequential (pool buffers
    # cycle per tile() call; interleaved live chains collide).  ps and psv
    # are two single-bank pools used as a manual double buffer: concurrent
    # chain families get one each, and op units alternate halves across both.
    def g_proj(which, pair, ts, pool):
        """q or k projection of one 512-token span for one head pair:
        8 accumulating matmuls + 1 copy-out, yielded as 9 thunks."""
        w_sb, dst = (wq_sb, qT) if which == "q" else (wk_sb, kT)
        tsl = slice(ts * 512, (ts + 1) * 512)
        st = {}

        def mk(fc):
            def th():
                if fc == 0:
                    st["p"] = pool.tile([128, 512], FP32, tag="proj", name="qk")
                nc.tensor.matmul(
                    st["p"][:],
                    lhsT=w_sb[:, fc, pair * 128:(pair + 1) * 128],
                    rhs=xt_sb[:, fc, tsl],
                    start=(fc == 0), stop=(fc == FC - 1))
            return th

        for fc in range(FC):
            yield mk(fc)

        def cp():
            nc.vector.tensor_copy(out=dst[:, pair, tsl], in_=st["p"][:])
        yield cp

    def g_vp2(pairlo, j, pool):
        """v-projection of two adjacent head pairs (256 v-dims) for TWO
        adjacent 128-token chunks (kc=2j,2j+1) sharing one PSUM bank as two
        SEQUENTIAL 8-deep accumulations + one combined copy.  Pairing
        halves the per-unit copy-WAR serialization events that
        head-of-line-block the in-order PE stream."""
        st = {}

        def mk(fc, half):
            def th():
                if fc == 0 and half == 0:
                    st["p"] = pool.tile([128, 512], FP32, tag="proj", name="vp")
                kc = 2 * j + half
                nc.tensor.matmul(
                    st["p"][:, half * 256:(half + 1) * 256],
                    lhsT=xt_sb[:, fc, kc * 128:(kc + 1) * 128],
                    rhs=wv_sb[:, fc, pairlo * 128:(pairlo + 2) * 128],
                    start=(fc == 0), stop=(fc == FC - 1))
            return th

        for half in (0, 1):
            for fc in range(FC):
                yield mk(fc, half)

        def cp():
            nc.vector.tensor_copy(
                out=v_sb[:, 2 * j:2 * j + 2, 2 * pairlo:2 * pairlo + 4, 0:64],
                in_=st["p"][:].rearrange("p (k h d) -> p k h d", k=2, h=4))
        yield cp

    op_half = [0]

    def g_op(q):
        """output projection of quarter q's tokens: 4 token chunks; per
        chunk the two 512-column halves accumulate in alternating ps/psv
        banks (ping-pong across the whole op stream) with their matmuls
        interleaved cc-major, so the cc<3 accumulation runs before the
        final quarter's norm lands and same-bank stalls are avoided."""
        st = {}

        def mk(u, j2, cc):
            def th():
                tcg = q * 4 + u
                if cc == 0:
                    if j2 == 0:
                        st["ot"] = osb.tile([128, D], BF16, tag="ot", name="ot")
                    op_half[0] += 1
                    st[j2] = (ps if op_half[0] % 2 == 0 else psv).tile(
                        [128, 512], FP32, tag="proj", name="po")
                nc.tensor.matmul(
                    st[j2][:],
                    lhsT=ctx_sb[:, cc, tcg * 128:(tcg + 1) * 128],
                    rhs=wo_sb[:, cc, j2 * 512:(j2 + 1) * 512],
                    start=(cc == 0), stop=(cc == 3))
            return th

        def mkcp(u, j2):
            def th():
                tcg = q * 4 + u
                nc.vector.tensor_copy(
                    out=st["ot"][:, j2 * 512:(j2 + 1) * 512], in_=st[j2][:])
                if j2 == 1:
                    eng = nc.scalar if (q == 3 and u % 2 == 1) else nc.sync
                    eng.dma_start(
                        out=out_d[tcg * 128:(tcg + 1) * 128, :],
                        in_=st["ot"][:])
            return th

        for u in range(4):
            for cc in range(3):
                for j2 in range(2):
                    yield mk(u, j2, cc)
            yield mk(u, 0, 3)
            yield mkcp(u, 0)
            yield mk(u, 1, 3)
            yield mkcp(u, 1)

    # ---------------- schedule: spread thunks over step ranges -------------
    sched = [[] for _ in range(NSTEP + 2)]

    def spread(thunks, s0, s1):
        thunks = list(thunks)
        n, span = len(thunks), s1 - s0
        for idx, th in enumerate(thunks):
            sched[s0 + (idx * span) // n].append(th)

    def chain(*gens):
        for g in gens:
            yield from g

    # ps chain, steps 0-16: k spans 1-3 of pair 0 (span s needed by step
    # 4s) then q span 1 (needed by step 16) — one sequential chain.
    spread(chain(g_proj("k", 0, 1, ps), g_proj("k", 0, 2, ps),
                 g_proj("k", 0, 3, ps), g_proj("q", 0, 1, ps)), 0, 16)
    # psv chain, steps 0-16: v pairs 0+1, chunk pair j (kc=2j,2j+1) due by
    # step 2j+2 (AV has ring-buffer slack if it slips a little)
    for j in range(KC // 2):
        spread(g_vp2(0, j, psv), 2 * j, 2 * j + 2)
    # ps chain, steps 16-64: pair-0 q spans 2,3 then all of pair 1's q/k
    spread(chain(g_proj("q", 0, 2, ps), g_proj("q", 0, 3, ps),
                 *[g_proj(w, 1, ts, ps) for w in ("k", "q") for ts in range(4)]),
           16, 64)
    # psv chain, steps 16-64: v pairs 2+3 (needed from step 131)
    spread(chain(*[g_vp2(2, j, psv) for j in range(KC // 2)]), 16, 64)
    # pair 2 / pair 3 q/k projections: units alternate ps/psv banks
    for base, pair in ((64, 2), (128, 3)):
        units = [g_proj(w, pair, ts, (ps if i % 2 == 0 else psv))
                 for i, (w, ts) in enumerate(
                     (w, ts) for w in ("k", "q") for ts in range(4))]
        spread(chain(*units), base, base + 64)
    # output projection of hc3's quarters 0-2 during quarters 1-3.
    # norm(3,q) is emitted at step 209+16q (with the AV of step a=207+16q),
    # so op(q) must not enter the PE stream before step 210+16q or it
    # head-of-line-blocks the PE behind a dependency on a later AV.
    for q in range(3):
        spread(g_op(q), 210 + 16 * q, min(226 + 16 * q, NSTEP))

    # ---------------- prologue projections (k span 0, q span 0, pair 0) ----
    # interleave the two chains (ps/psv banks) to avoid same-bank
    # accumulation stalls in the serial prologue
    for thk, thq in zip(g_proj("k", 0, 0, ps), g_proj("q", 0, 0, psv)):
        thk()
        thq()

    # ---------------- normalization at quarter end -------------------------
    def norm_q(pc, qq, ctxA, ctxB):
        qsl = slice(qq * 512, (qq + 1) * 512)
        # fast copies free the ctx PSUM banks for the next quarter's AV; the
        # Z rows must land at partition 0 (custom-DVE reciprocal reads the
        # input AP without its partition offset).
        sc = scr.tile([64, 2, 512], FP32, tag="scr", name="scr")
        zc = rpool.tile([1, 2, 512], FP32, tag="zc", name="zc")
        nc.vector.tensor_copy(out=sc[:, 0, :], in_=ctxA[0:64, :])
        nc.vector.tensor_copy(out=zc[:, 0, :], in_=ctxA[64:65, :])
        nc.vector.tensor_copy(out=sc[:, 1, :], in_=ctxB[0:64, :])
        nc.vector.tensor_copy(out=zc[:, 1, :], in_=ctxB[64:65, :])
        rt = rpool.tile([1, 2, 512], FP32, tag="rt", name="rt")
        nc.vector.reciprocal_approx_fast(out=rt[:], in_=zc[:])
        for h in (0, 1):
            rb = rpool.tile([64, 512], FP32, tag="rb")
            nc.gpsimd.partition_broadcast(rb[:], rt[0:1, h, :], channels=64)
            nc.vector.tensor_mul(
                ctx_sb[h * 64:h * 64 + 64, pc, qsl], sc[:, h, :], rb[:])

    # ---------------- main pipeline ----------------------------------------
    sps_hist = {}
    ctxA = ctxB = None
    for i in range(NSTEP + 2):
        if i < NSTEP:
            pc, r = divmod(i, QQ * KC)
            qq, kc = divmod(r, KC)
            qsl = slice(qq * 512, (qq + 1) * 512)
            sps = spsum.tile([128, 2, 512], FP32, tag="S", name="S")
            for h in (0, 1):
                b0 = h * 64
                nc.tensor.matmul(
                    sps[:, h, :],
                    lhsT=kT[b0:b0 + 64, pc, kc * 128:(kc + 1) * 128],
                    rhs=qT[b0:b0 + 64, pc, qsl],
                    start=True, stop=True)
            sps_hist[i] = sps
        j = i - 1
        if 0 <= j < NSTEP:
            nc.scalar.activation(
                out=P2[:, j % RING, :, :], in_=sps_hist.pop(j)[:, :, :],
                func=EXP, scale=0.125)
        for th in sched[i]:
            th()
        a = i - 2
        if 0 <= a < NSTEP:
            apc, ar = divmod(a, QQ * KC)
            aqq, akc = divmod(ar, KC)
            if akc == 0:
                ctxA = cpsum.tile([65, 512], FP32, tag="ctx", name="cA")
                ctxB = cpsum.tile([65, 512], FP32, tag="ctx", name="cB")
            for hi, ctp in ((0, ctxA), (1, ctxB)):
                nc.tensor.matmul(
                    ctp[:],
                    lhsT=v_sb[:, akc, 2 * apc + hi, :],
                    rhs=P2[:, a % RING, hi, :],
                    start=(akc == 0), stop=(akc == KC - 1))
            if akc == KC - 1:
                norm_q(apc, aqq, ctxA, ctxB)

    # ---------------- tail: output projection of the last quarter ----------
    for th in g_op(3):
        th()
    return qT, kT, v_sb, ctx_sb


def build():
    nc = bacc.Bacc("TRN2", target_bir_lowering=False, debug=False, num_devices=8)
    xt_d = nc.dram_tensor("xt", [D, T], BF16, kind="ExternalInput").ap()
    wq_d = nc.dram_tensor("wq", [D, 512], BF16, kind="ExternalInput").ap()
    wk_d = nc.dram_tensor("wk", [D, 512], BF16, kind="ExternalInput").ap()
    wv_d = nc.dram_tensor("wv", [D, 512], BF16, kind="ExternalInput").ap()
    wo_d = nc.dram_tensor("wout", [512, D], BF16, kind="ExternalInput").ap()
    out_d = nc.dram_tensor("out", [T, D], BF16, kind="ExternalOutput").ap()
    with tile.TileContext(nc) as tc:
        with ExitStack() as ctx:
            _body(ctx, nc, tc, xt_d, wq_d, wk_d, wv_d, wo_d, out_d)
    nc.compile()
    return nc


_nc = None


def _get_nc():
    global _nc
    if _nc is None:
        _nc = build()
    return _nc


def make_in_maps(x, Wqkv, Wout):
    bf = ml_dtypes.bfloat16
    in_maps = []
    for c in range(8):
        b, g = divmod(c, 2)
        cs = slice(g * 512, (g + 1) * 512)
        in_maps.append({
            "xt": np.ascontiguousarray(x[b].T).astype(bf),
            "wq": np.ascontiguousarray(Wqkv[:, 0 * D:1 * D][:, cs]).astype(bf),
            "wk": np.ascontiguousarray(Wqkv[:, 1 * D:2 * D][:, cs]).astype(bf),
            "wv": np.ascontiguousarray(Wqkv[:, 2 * D:3 * D][:, cs]).astype(bf),
            "wout": np.ascontiguousarray(Wout[cs, :]).astype(bf),
        })
    return in_maps


def kernel(x, Wqkv, Wout, _trace=False):
    nc = _get_nc()
    x = np.asarray(x, dtype=np.float32)
    Wqkv = np.asarray(Wqkv, dtype=np.float32)
    Wout = np.asarray(Wout, dtype=np.float32)
    in_maps = make_in_maps(x, Wqkv, Wout)
    kwargs = {}
    if _trace:
        kwargs["trace"] = True
    res = run_bass_kernel_spmd(nc, in_maps, core_ids=list(range(8)), **kwargs)
    outs = [np.asarray(res.results[c]["out"], dtype=np.float32) for c in range(8)]
    out = np.stack([outs[2 * b] + outs[2 * b + 1] for b in range(4)])
    if _trace:
        kernel.last_result = res
    return out


# revision 5
# speedup vs baseline: 1.0054x; 1.0037x over previous
"""Multi-head attention (B=4, T=2048, D=1024, H=16) on 8 TRN2 NeuronCores.

Sharding: core c -> (batch b = c//2, head-group g = c%2 of 8 heads).
Each core computes the qkv projection for its batch restricted to its 8
heads, full attention for those heads, and a partial output projection
(ctx_local @ Wout[rows of its heads]).  Host sums the two partials per batch.

v2: flat software pipeline.  x is resident in SBUF (loaded once).  The
attention runs as one uniform 256-step stream over (pair, quarter, kchunk):
per step  [S-pair(i) | exp(i-1) on ACT | extra proj work | AV-pair(i-2)],
with all projection matmuls (q/k/v/out) spread into the per-step "extra"
slots so ACT (the softmax-exp engine, the throughput floor at ~1.1us per
[128,1024] exp) stays saturated from ~10us onward.  Startup DMAs are
chunked (x span 0 + pair-0 weights first, on two HWDGE queues) so the
first S matmul issues at ~9us.  Quarter-end ctx PSUM banks are freed by a
fast DVE copy to SBUF scratch before the (slow) reciprocal+broadcast
normalization, so the next quarter's AV never stalls on the norm chain.
"""

import numpy as np
import ml_dtypes
from contextlib import ExitStack

import concourse.bass as bass
import concourse.bacc as bacc
import concourse.tile as tile
from concourse import mybir
from concourse.bass_utils import run_bass_kernel_spmd

FP32 = mybir.dt.float32
BF16 = mybir.dt.bfloat16
EXP = mybir.ActivationFunctionType.Exp

D = 1024
T = 2048
FC = 8      # feature chunks of 128 (projection contraction)
KC = 16     # k chunks of 128
QQ = 4      # query quarters of 512
NPAIR = 4   # head pairs per core
RING = 8    # P2 ring depth (k-chunk slots)
NSTEP = NPAIR * QQ * KC  # 256


def _body(ctx, nc, tc, xt_d, wq_d, wk_d, wv_d, wo_d, out_d):
    per = ctx.enter_context(tc.tile_pool(name="persist", bufs=1))
    xt_sb = per.tile([128, FC, T], BF16, tag="xt")
    qT = per.tile([128, NPAIR, T], BF16, tag="qT")
    kT = per.tile([128, NPAIR, T], BF16, tag="kT")
    ctx_sb = per.tile([128, NPAIR, T], BF16, tag="ctx")
    v_sb = per.tile([128, KC, 8, 65], BF16, tag="v")
    wq_sb = per.tile([128, FC, 512], BF16, tag="wq")
    wk_sb = per.tile([128, FC, 512], BF16, tag="wk")
    wv_sb = per.tile([128, FC, 512], BF16, tag="wv")
    wo_sb = per.tile([128, 4, D], BF16, tag="wo")
    P2 = per.tile([128, RING, 2, 512], BF16, tag="P2")
    warm_in = per.tile([1, 16], FP32, tag="warm_in")
    warm_out = per.tile([1, 16], FP32, tag="warm_out")

    xt_r = xt_d.rearrange("(f p) t -> p f t", p=128)
    wq_r = wq_d.rearrange("(f p) c -> p f c", p=128)
    wk_r = wk_d.rearrange("(f p) c -> p f c", p=128)
    wv_r = wv_d.rearrange("(f p) c -> p f c", p=128)
    wo_r = wo_d.rearrange("(c p) d -> p c d", p=128)

    # --- prologue DMAs.  sync queue: x spans + background weights;
    # scalar queue (ACT is idle until the first exp): pair-0 q/k weights.
    nc.sync.dma_start(out=xt_sb[:, :, 0:512], in_=xt_r[:, :, 0:512])
    nc.scalar.dma_start(out=wk_sb[:, :, 0:128], in_=wk_r[:, :, 0:128])
    nc.scalar.dma_start(out=wq_sb[:, :, 0:128], in_=wq_r[:, :, 0:128])
    nc.sync.dma_start(out=wv_sb[:, :, 0:128], in_=wv_r[:, :, 0:128])
    nc.scalar.dma_start(out=xt_sb[:, :, 1536:2048], in_=xt_r[:, :, 1536:2048])
    for s in range(1, 3):
        nc.sync.dma_start(
            out=xt_sb[:, :, s * 512:(s + 1) * 512],
            in_=xt_r[:, :, s * 512:(s + 1) * 512])
    nc.sync.dma_start(out=wk_sb[:, :, 128:512], in_=wk_r[:, :, 128:512])
    nc.sync.dma_start(out=wq_sb[:, :, 128:512], in_=wq_r[:, :, 128:512])
    nc.sync.dma_start(out=wv_sb[:, :, 128:512], in_=wv_r[:, :, 128:512])
    nc.sync.dma_start(out=wo_sb[:], in_=wo_r)

    # ones row for the fused sumexp; warm the ACT exp table during the DMAs
    nc.vector.memset(v_sb[:, :, :, 64:65], 1.0)
    nc.vector.memset(warm_in[:], 0.0)
    nc.scalar.activation(out=warm_out[:], in_=warm_in[:], func=EXP, scale=1.0)

    ps = ctx.enter_context(tc.tile_pool(name="ps", bufs=1, space="PSUM"))
    psv = ctx.enter_context(tc.tile_pool(name="psv", bufs=1, space="PSUM"))
    spsum = ctx.enter_context(tc.tile_pool(name="sps", bufs=2, space="PSUM"))
    cpsum = ctx.enter_context(tc.tile_pool(name="cps", bufs=2, space="PSUM"))
    scr = ctx.enter_context(tc.tile_pool(name="scr", bufs=2))
    rpool = ctx.enter_context(tc.tile_pool(name="rp", bufs=2))
    osb = ctx.enter_context(tc.tile_pool(name="osb", bufs=2))

    # ---------------- extra-work generators (thunks emit on call) ----------
    # Chains that share a PSUM pool must be strictly sequential (pool buffers
    # cycle per tile() call; interleaved live chains collide).  ps and psv
    # are two single-bank pools used as a manual double buffer: concurrent
    # chain families get one each, and op units alternate halves across both.
    def g_proj(which, pair, ts, pool):
        """q or k projection of one 512-token span for one head pair:
        8 accumulating matmuls + 1 copy-out, yielded as 9 thunks."""
        w_sb, dst = (wq_sb, qT) if which == "q" else (wk_sb, kT)
        tsl = slice(ts * 512, (ts + 1) * 512)
        st = {}

        def mk(fc):
            def th():
                if fc == 0:
                    st["p"] = pool.tile([128, 512], FP32, tag="proj", name="qk")
                nc.tensor.matmul(
                    st["p"][:],
                    lhsT=w_sb[:, fc, pair * 128:(pair + 1) * 128],
                    rhs=xt_sb[:, fc, tsl],
                    start=(fc == 0), stop=(fc == FC - 1))
            return th

        for fc in range(FC):
            yield mk(fc)

        def cp():
            nc.vector.tensor_copy(out=dst[:, pair, tsl], in_=st["p"][:])
        yield cp

    def g_vp2(pairlo, j, pool):
        """v-projection of two adjacent head pairs (256 v-dims) for TWO
        adjacent 128-token chunks (kc=2j,2j+1) sharing one PSUM bank as two
        SEQUENTIAL 8-deep accumulations + one combined copy.  Pairing
        halves the per-unit copy-WAR serialization events that
        head-of-line-block the in-order PE stream."""
        st = {}

        def mk(fc, half):
            def th():
                if fc == 0 and half == 0:
                    st["p"] = pool.tile([128, 512], FP32, tag="proj", name="vp")
                kc = 2 * j + half
                nc.tensor.matmul(
                    st["p"][:, half * 256:(half + 1) * 256],
                    lhsT=xt_sb[:, fc, kc * 128:(kc + 1) * 128],
                    rhs=wv_sb[:, fc, pairlo * 128:(pairlo + 2) * 128],
                    start=(fc == 0), stop=(fc == FC - 1))
            return th

        for half in (0, 1):
            for fc in range(FC):
                yield mk(fc, half)

        def cp():
            nc.vector.tensor_copy(
                out=v_sb[:, 2 * j:2 * j + 2, 2 * pairlo:2 * pairlo + 4, 0:64],
                in_=st["p"][:].rearrange("p (k h d) -> p k h d", k=2, h=4))
        yield cp

    op_half = [0]

    def g_op(q):
        """output projection of quarter q's tokens: 4 token chunks; per
        chunk the two 512-column halves accumulate in alternating ps/psv
        banks (ping-pong across the whole op stream) with their matmuls
        interleaved cc-major, so the cc<3 accumulation runs before the
        final quarter's norm lands and same-bank stalls are avoided."""
        st = {}

        def mk(u, j2, cc):
            def th():
                tcg = q * 4 + u
                if cc == 0:
                    if j2 == 0:
                        st["ot"] = osb.tile([128, D], BF16, tag="ot", name="ot")
                    op_half[0] += 1
                    st[j2] = (ps if op_half[0] % 2 == 0 else psv).tile(
                        [128, 512], FP32, tag="proj", name="po")
                nc.tensor.matmul(
                    st[j2][:],
                    lhsT=ctx_sb[:, cc, tcg * 128:(tcg + 1) * 128],
                    rhs=wo_sb[:, cc, j2 * 512:(j2 + 1) * 512],
                    start=(cc == 0), stop=(cc == 3))
            return th

        def mkcp(u, j2):
            def th():
                tcg = q * 4 + u
                nc.vector.tensor_copy(
                    out=st["ot"][:, j2 * 512:(j2 + 1) * 512], in_=st[j2][:])
                if j2 == 1:
                    eng = nc.scalar if (q == 3 and u % 2 == 1) else nc.sync
                    eng.dma_start(
                        out=out_d[tcg * 128:(tcg + 1) * 128, :],
                        in_=st["ot"][:])
            return th

        for u in range(4):
            for cc in range(3):
                for j2 in range(2):
                    yield mk(u, j2, cc)
            yield mk(u, 0, 3)
            yield mkcp(u, 0)
            yield mk(u, 1, 3)
            yield mkcp(u, 1)

    # ---------------- schedule: spread thunks over step ranges -------------
    sched = [[] for _ in range(NSTEP + 2)]

    def spread(thunks, s0, s1):
        thunks = list(thunks)
        n, span = len(thunks), s1 - s0
        for idx, th in enumerate(thunks):
            sched[s0 + (idx * span) // n].append(th)

    def chain(*gens):
        for g in gens:
            yield from g

    # ps chain, steps 0-16: k spans 1-3 of pair 0 (span s needed by step
    # 4s) then q span 1 (needed by step 16) — one sequential chain.
    spread(chain(g_proj("k", 0, 1, ps), g_proj("k", 0, 2, ps),
                 g_proj("k", 0, 3, ps), g_proj("q", 0, 1, ps)), 0, 16)
    # psv chain, steps 0-16: v pairs 0+1, chunk pair j (kc=2j,2j+1) due by
    # step 2j+2 (AV has ring-buffer slack if it slips a little)
    for j in range(KC // 2):
        spread(g_vp2(0, j, psv), 2 * j, 2 * j + 2)
    # steps 16-64: ONE merged chain alternating qk units (ps) with paired
    # vp23 units (psv) — each bank's next user is two units (~3.5us) after
    # its copy-out, so the per-unit copy-WAR stall vanishes.  All deadlines
    # here are loose (q2 due step 32, q3 due 48, pair-1 qk due 64, vp23 due
    # 131), so the coupling is safe (unlike qq0, where k spans are urgent).
    qk_list = ([("q", 0, 2), ("q", 0, 3)] +
               [(w, 1, ts) for w in ("k", "q") for ts in range(4)])
    mid_units = []
    for idx in range(18):
        pool = ps if idx % 2 == 0 else psv
        if idx < 16:
            if idx % 2 == 0:
                w, pr, ts = qk_list[idx // 2]
                mid_units.append(g_proj(w, pr, ts, pool))
            else:
                mid_units.append(g_vp2(2, idx // 2, pool))
        else:
            w, pr, ts = qk_list[8 + idx - 16]
            mid_units.append(g_proj(w, pr, ts, pool))
    spread(chain(*mid_units), 16, 64)
    # pair 2 / pair 3 q/k projections: units alternate ps/psv banks
    for base, pair in ((64, 2), (128, 3)):
        units = [g_proj(w, pair, ts, (ps if i % 2 == 0 else psv))
                 for i, (w, ts) in enumerate(
                     (w, ts) for w in ("k", "q") for ts in range(4))]
        spread(chain(*units), base, base + 64)
    # output projection of hc3's quarters 0-2 during quarters 1-3.
    # norm(3,q) is emitted at step 209+16q (with the AV of step a=207+16q),
    # so op(q) must not enter the PE stream before step 210+16q or it
    # head-of-line-blocks the PE behind a dependency on a later AV.
    for q in range(3):
        spread(g_op(q), 210 + 16 * q, min(226 + 16 * q, NSTEP))

    # ---------------- prologue projections (k span 0, q span 0, pair 0) ----
    # interleave the two chains (ps/psv banks) to avoid same-bank
    # accumulation stalls in the serial prologue
    for thk, thq in zip(g_proj("k", 0, 0, ps), g_proj("q", 0, 0, psv)):
        thk()
        thq()

    # ---------------- normalization at quarter end -------------------------
    def norm_q(pc, qq, ctxA, ctxB):
        qsl = slice(qq * 512, (qq + 1) * 512)
        # fast copies free the ctx PSUM banks for the next quarter's AV; the
        # Z rows must land at partition 0 (custom-DVE reciprocal reads the
        # input AP without its partition offset).
        sc = scr.tile([64, 2, 512], FP32, tag="scr", name="scr")
        zc = rpool.tile([1, 2, 512], FP32, tag="zc", name="zc")
        nc.vector.tensor_copy(out=sc[:, 0, :], in_=ctxA[0:64, :])
        nc.vector.tensor_copy(out=zc[:, 0, :], in_=ctxA[64:65, :])
        nc.vector.tensor_copy(out=sc[:, 1, :], in_=ctxB[0:64, :])
        nc.vector.tensor_copy(out=zc[:, 1, :], in_=ctxB[64:65, :])
        rt = rpool.tile([1, 2, 512], FP32, tag="rt", name="rt")
        nc.vector.reciprocal_approx_fast(out=rt[:], in_=zc[:])
        for h in (0, 1):
            rb = rpool.tile([64, 512], FP32, tag="rb")
            nc.gpsimd.partition_broadcast(rb[:], rt[0:1, h, :], channels=64)
            nc.vector.tensor_mul(
                ctx_sb[h * 64:h * 64 + 64, pc, qsl], sc[:, h, :], rb[:])

    # ---------------- main pipeline ----------------------------------------
    sps_hist = {}
    ctxA = ctxB = None
    for i in range(NSTEP + 2):
        if i < NSTEP:
            pc, r = divmod(i, QQ * KC)
            qq, kc = divmod(r, KC)
            qsl = slice(qq * 512, (qq + 1) * 512)
            sps = spsum.tile([128, 2, 512], FP32, tag="S", name="S")
            for h in (0, 1):
                b0 = h * 64
                nc.tensor.matmul(
                    sps[:, h, :],
                    lhsT=kT[b0:b0 + 64, pc, kc * 128:(kc + 1) * 128],
                    rhs=qT[b0:b0 + 64, pc, qsl],
                    start=True, stop=True)
            sps_hist[i] = sps
        j = i - 1
        if 0 <= j < NSTEP:
            nc.scalar.activation(
                out=P2[:, j % RING, :, :], in_=sps_hist.pop(j)[:, :, :],
                func=EXP, scale=0.125)
        for th in sched[i]:
            th()
        a = i - 2
        if 0 <= a < NSTEP:
            apc, ar = divmod(a, QQ * KC)
            aqq, akc = divmod(ar, KC)
            if akc == 0:
                ctxA = cpsum.tile([65, 512], FP32, tag="ctx", name="cA")
                ctxB = cpsum.tile([65, 512], FP32, tag="ctx", name="cB")
            for hi, ctp in ((0, ctxA), (1, ctxB)):
                nc.tensor.matmul(
                    ctp[:],
                    lhsT=v_sb[:, akc, 2 * apc + hi, :],
                    rhs=P2[:, a % RING, hi, :],
                    start=(akc == 0), stop=(akc == KC - 1))
            if akc == KC - 1:
                norm_q(apc, aqq, ctxA, ctxB)

    # ---------------- tail: output projection of the last quarter ----------
    for th in g_op(3):
        th()
    return qT, kT, v_sb, ctx_sb


def build():
    nc = bacc.Bacc("TRN2", target_bir_lowering=False, debug=False, num_devices=8)
    xt_d = nc.dram_tensor("xt", [D, T], BF16, kind="ExternalInput").ap()
    wq_d = nc.dram_tensor("wq", [D, 512], BF16, kind="ExternalInput").ap()
    wk_d = nc.dram_tensor("wk", [D, 512], BF16, kind="ExternalInput").ap()
    wv_d = nc.dram_tensor("wv", [D, 512], BF16, kind="ExternalInput").ap()
    wo_d = nc.dram_tensor("wout", [512, D], BF16, kind="ExternalInput").ap()
    out_d = nc.dram_tensor("out", [T, D], BF16, kind="ExternalOutput").ap()
    with tile.TileContext(nc) as tc:
        with ExitStack() as ctx:
            _body(ctx, nc, tc, xt_d, wq_d, wk_d, wv_d, wo_d, out_d)
    nc.compile()
    return nc


_nc = None


def _get_nc():
    global _nc
    if _nc is None:
        _nc = build()
    return _nc


def make_in_maps(x, Wqkv, Wout):
    bf = ml_dtypes.bfloat16
    in_maps = []
    for c in range(8):
        b, g = divmod(c, 2)
        cs = slice(g * 512, (g + 1) * 512)
        in_maps.append({
            "xt": np.ascontiguousarray(x[b].T).astype(bf),
            "wq": np.ascontiguousarray(Wqkv[:, 0 * D:1 * D][:, cs]).astype(bf),
            "wk": np.ascontiguousarray(Wqkv[:, 1 * D:2 * D][:, cs]).astype(bf),
            "wv": np.ascontiguousarray(Wqkv[:, 2 * D:3 * D][:, cs]).astype(bf),
            "wout": np.ascontiguousarray(Wout[cs, :]).astype(bf),
        })
    return in_maps


def kernel(x, Wqkv, Wout, _trace=False):
    nc = _get_nc()
    x = np.asarray(x, dtype=np.float32)
    Wqkv = np.asarray(Wqkv, dtype=np.float32)
    Wout = np.asarray(Wout, dtype=np.float32)
    in_maps = make_in_maps(x, Wqkv, Wout)
    kwargs = {}
    if _trace:
        kwargs["trace"] = True
    res = run_bass_kernel_spmd(nc, in_maps, core_ids=list(range(8)), **kwargs)
    outs = [np.asarray(res.results[c]["out"], dtype=np.float32) for c in range(8)]
    out = np.stack([outs[2 * b] + outs[2 * b + 1] for b in range(4)])
    if _trace:
        kernel.last_result = res
    return out
